# revision 1
# baseline (speedup 1.0000x reference)
# Trainium2 Bass kernel for nn_CrossAttentionLayer (linear attention with
# elu+1 feature map).
#
# Math (per batch n):
#   q = guidance @ Wq.T + bq ; k = x @ Wk.T + bk ; v = x @ Wv.T + bv
#   Q = elu(q)+1 ; K = elu(k)+1          (per head h, head dim D=64)
#   KV_h = K_h^T @ (v_h/S);  Z = 1/(Q_h . sum_s K_h + eps)
#   out_h = (Q_h @ KV_h) * Z * S         (the /S and *S cancel exactly)
#
# Sharding: 8 cores = batch(4) x guidance-halves(2). Each core recomputes
# K/V/KV/Ksum for its batch over the full source sequence S=4096 (dup x2),
# and the Q side for its 2048 guidance rows.
#
# On-chip dataflow (all matmuls in float32r: full-rate PE fp32):
#  phase 1 (per 128-row s-tile of x):
#    PE-transpose x-tile -> xT ; k/v projections token-major with xT as
#    stationary (bias added via a K=1 matmul against a ones row);
#    K = elu(k)+1 computed as max(relu(k+1), min(exp(k), 1));
#    KV accumulated in PSUM via 4 two-head matmuls (N=256); Ksum via a
#    ones-column matmul (N=512).
#  phase 2 (per 512-row l-chunk of guidance):
#    PE-transpose g -> gT ; q projection FEATURE-major (weights stationary,
#    per-partition bias via the activation); denominators via block-diagonal
#    Ksum matrix; out = (Q @ blockdiag(KV)) scaled by Z broadcast.

import sys

import numpy as np

if "/opt/trn_rl_repo" not in sys.path:
    sys.path.insert(0, "/opt/trn_rl_repo")

import concourse.bacc as bacc
import concourse.mybir as mybir
import concourse.tile as tile
from concourse import bass_utils
from concourse.masks import make_identity

P = 128
S = 4096
LC = 2048  # guidance rows per core
C = 512
H = 8
D = 64
NCT = C // P  # 4 column tiles
NST = S // P  # 32 s-tiles
EPS = 1e-6
SPLIT_S = True  # s-split across core pairs + AllReduce of partial KV/Ksum
SKIP_CC = False  # timing experiment: skip the collective (WRONG results)

F32 = mybir.dt.float32
F32R = mybir.dt.float32r

Exp = mybir.ActivationFunctionType.Exp
Relu = mybir.ActivationFunctionType.Relu


def _build_nc(reps=1, with_bias=False, split_s=None):
    if split_s is None:
        split_s = SPLIT_S
    nc = bacc.Bacc(
        "TRN2",
        target_bir_lowering=False,
        debug=False,
        enable_asserts=False,
        num_devices=8,
    )
    xs = S // 2 if split_s else S
    xb = nc.dram_tensor("xb", [xs, C], F32, kind="ExternalInput").ap()
    gb = nc.dram_tensor("gb", [LC, C], F32, kind="ExternalInput").ap()
    wkt = nc.dram_tensor("wkt", [C, C], F32, kind="ExternalInput").ap()
    wvt = nc.dram_tensor("wvt", [C, C], F32, kind="ExternalInput").ap()
    wqt = nc.dram_tensor("wqt", [C, C], F32, kind="ExternalInput").ap()
    bk = nc.dram_tensor("bk", [C], F32, kind="ExternalInput").ap()
    bv = nc.dram_tensor("bv", [C], F32, kind="ExternalInput").ap()
    bq = nc.dram_tensor("bq", [C], F32, kind="ExternalInput").ap()
    outb = nc.dram_tensor("outb", [LC, C], F32, kind="ExternalOutput").ap()

    with tile.TileContext(nc) as tc:
        for rep in range(reps):
            _emit(nc, tc, xb, gb, wkt, wvt, wqt, bk, bv, bq, outb, rep=rep,
                  with_bias=with_bias, split_s=split_s)

    nc.compile()
    return nc


def _emit(nc, tc, xb, gb, wkt, wvt, wqt, bk, bv, bq, outb, rep=0,
          with_bias=False, split_s=False):
    mm = nc.tensor.matmul
    nst = NST // 2 if split_s else NST
    with (
        tc.tile_pool(name=f"persist{rep}", bufs=1) as pp,
        tc.tile_pool(name=f"dram{rep}", bufs=1, space="DRAM") as dp,
    ):
        # --- constants / weights resident in SBUF ---
        # fp32r matmul operands must be produced by DVE/ACT compute ops (the
        # verifier requires an explicit rounding producer), so DMA/memset
        # results are staged in fp32 and copied into fp32r tiles on DVE.
        wk_sb = pp.tile([P, NCT, C], F32R)
        wv_sb = pp.tile([P, NCT, C], F32R)
        wq_sb = pp.tile([P, NCT, C], F32R)
        bk_row = pp.tile([1, C], F32R)
        bv_row = pp.tile([1, C], F32R)
        ones_row = pp.tile([1, P], F32R)
        ones_colr = pp.tile([P, 1], F32R)
        ident = pp.tile([P, P], F32)
        make_identity(nc, ident)
        if True:
            ip = pp
            wk_st = ip.tile([P, NCT, C], F32, name="wk_st")
            wv_st = ip.tile([P, NCT, C], F32, name="wv_st")
            wq_st = ip.tile([P, NCT, C], F32, name="wq_st")
            nc.gpsimd.dma_start(wk_st, wkt.rearrange("(t p) n -> p t n", p=P))
            nc.gpsimd.dma_start(wv_st, wvt.rearrange("(t p) n -> p t n", p=P))
            nc.gpsimd.dma_start(wq_st, wqt.rearrange("(t p) n -> p t n", p=P))
            nc.vector.tensor_copy(wk_sb, wk_st)
            nc.vector.tensor_copy(wv_sb, wv_st)
            nc.vector.tensor_copy(wq_sb, wq_st)
            bk_st = ip.tile([1, C], F32, name="bk_st")
            bv_st = ip.tile([1, C], F32, name="bv_st")
            nc.gpsimd.dma_start(bk_st, bk.rearrange("(a c) -> a c", a=1))
            nc.gpsimd.dma_start(bv_st, bv.rearrange("(a c) -> a c", a=1))
            nc.vector.tensor_copy(bk_row, bk_st)
            nc.vector.tensor_copy(bv_row, bv_st)
            ones_st = ip.tile([1, P], F32, name="ones_st")
            nc.vector.memset(ones_st, 1.0)
            nc.vector.tensor_copy(ones_row, ones_st)
            onescol_st = ip.tile([P, 1], F32, name="onescol_st")
            nc.vector.memset(onescol_st, 1.0)
            nc.vector.tensor_copy(ones_colr, onescol_st)
        bqT = pp.tile([P, NCT], F32)
        nc.sync.dma_start(bqT, bq.rearrange("(t p) -> p t", p=P))
        bqT1 = pp.tile([P, NCT], F32)
        nc.vector.tensor_scalar_add(bqT1, bqT, 1.0)

        zero_col = pp.tile([P, 1], F32)
        nc.vector.memset(zero_col, 0.0)
        ones_col = pp.tile([P, 1], F32)
        nc.vector.memset(ones_col, 1.0)
        onezero = pp.tile([P, 2], F32)
        nc.vector.memset(onezero[:, 0:1], 1.0)
        nc.vector.memset(onezero[:, 1:2], 0.0)

        # blockdiag(KV_h) as [cin_part, cin_tile, C] and blockdiag Ksum
        kvbd = pp.tile([P, NCT, C], F32R)
        nc.vector.tensor_copy(kvbd, zero_col[:, :, None].to_broadcast([P, NCT, C]))
        ksbd = pp.tile([P, NCT, H], F32R)
        nc.vector.tensor_copy(ksbd, zero_col[:, :, None].to_broadcast([P, NCT, H]))
        ksumT = pp.tile([P, NCT], F32)

        # ---------------- phase 1: x -> K,V -> KV, Ksum ----------------
        # Each KV accumulation group owns a full PSUM bank (start=True zeroes
        # the whole 2KB zero region). The V operand carries an extra ones
        # column so column 256 of each KV psum accumulates Ksum directly in
        # feature-major layout.
        with (
            tc.tile_pool(name=f"p1_{rep}", bufs=3) as p1,
            tc.tile_pool(name=f"p1ps_{rep}", bufs=3, space="PSUM") as p1ps,
            tc.tile_pool(name=f"tps_{rep}", bufs=2, space="PSUM") as tps,
            tc.tile_pool(name=f"accps_{rep}", bufs=1, space="PSUM") as accps,
        ):
            kv_ps = [
                accps.tile([P, 2, 256], F32, tag=f"kv{b}", name=f"kv_ps{b}")
                for b in range(2)
            ]
            ksum_ps = accps.tile([1, C], F32, name="ksum_ps")

            def kv_mms(kv, first, last):
                # KV accumulation: two K-heads vs four V-heads per matmul.
                # Two accumulation groups share each PSUM bank: only the
                # first matmul into a bank uses start=True (it zeroes the
                # whole 2KB zero region), only the last uses stop=True.
                k_sb, v_ext = kv
                for hh in range(4):
                    mm(kv_ps[hh // 2][:, hh % 2, :],
                       k_sb[:, hh * P : (hh + 1) * P],
                       v_ext[:, hh // 2, :],
                       start=(first and hh % 2 == 0),
                       stop=(last and hh % 2 == 1))
                mm(ksum_ps, ones_colr, k_sb, start=first, stop=last)

            def consume(stage, first, last):
                # V split into two 4-head halves, each with a ones column
                # (and a zero pad column: fp32r matmuls need an even free dim)
                pk, pv = stage
                # K = elu(k)+1 = max(relu(k+1), min(exp(k), 1))
                e_sb = p1.tile([P, C], F32, tag="e")
                nc.scalar.activation(e_sb, pk, Exp)
                u_sb = p1.tile([P, C], F32, tag="u")
                nc.scalar.activation(u_sb, pk, Relu, bias=1.0)
                nc.vector.tensor_scalar_min(e_sb, e_sb, 1.0)
                k_sb = p1.tile([P, C], F32R, tag="k")
                nc.vector.tensor_tensor(k_sb, e_sb, u_sb, mybir.AluOpType.max)
                v_ext = p1.tile([P, 2, 256], F32R, tag="v")
                nc.scalar.copy(v_ext[:, 0, :], pv[:, 0:256])
                nc.scalar.copy(v_ext[:, 1, :], pv[:, 256:512])
                kv_mms((k_sb, v_ext), first, last)

            prev_stage = None
            for st in range(nst):
                xt = p1.tile([P, C], F32, tag="xt")
                nc.sync.dma_start(xt, xb[st * P : (st + 1) * P, :])
                xT = p1.tile([P, NCT, P], F32R, tag="xT")
                pt = tps.tile([P, NCT, P], F32, tag="tp")
                for ci in range(NCT):
                    mm(pt[:, ci, :], xt[:, ci * P : (ci + 1) * P], ident,
                       is_transpose=True,
                       start=(ci == 0), stop=(ci == NCT - 1))
                nc.vector.tensor_copy(xT, pt)
                # k projection (token-major): psum[s,cout]
                pk = p1ps.tile([P, C], F32, tag="proj")
                if with_bias:
                    mm(pk, ones_row, bk_row, start=True, stop=False)
                for ci in range(NCT):
                    mm(pk, xT[:, ci, :], wk_sb[:, ci, :],
                       start=(ci == 0 and not with_bias),
                       stop=(ci == NCT - 1))
                # v projection
                pv = p1ps.tile([P, C], F32, tag="proj")
                if with_bias:
                    mm(pv, ones_row, bv_row, start=True, stop=False)
                for ci in range(NCT):
                    mm(pv, xT[:, ci, :], wv_sb[:, ci, :],
                       start=(ci == 0 and not with_bias),
                       stop=(ci == NCT - 1))
                # software pipeline: consume the PREVIOUS iteration's psum
                # (elu + KV matmuls) so no engine queue ever heads-of-line
                # blocks this iteration's PE feed chain
                if prev_stage is not None:
                    consume(prev_stage, st == 1, False)
                prev_stage = (pk, pv)
            consume(prev_stage, False, True)

            # Ksum [1, C] -> feature-major [128, 4] via a DRAM round-trip
            ksum_row = pp.tile([1, C], F32)
            nc.vector.tensor_copy(ksum_row, ksum_ps)
            scratch = dp.tile([1, C], F32, name="scratch")
            nc.sync.dma_start(scratch, ksum_row)
            nc.sync.dma_start(
                ksumT, scratch.rearrange("a (t p) -> (a p) t", p=P)
            )
            if split_s:
                # pack partial KV banks + KsumT, AllReduce across the core
                # pair sharing this batch, then unpack the full sums
                stg = pp.tile([P, 1028], F32)
                nc.vector.tensor_copy(
                    stg[:, 0:512].rearrange("p (a v) -> p a v", a=2),
                    kv_ps[0],
                )
                nc.vector.tensor_copy(
                    stg[:, 512:1024].rearrange("p (a v) -> p a v", a=2),
                    kv_ps[1],
                )
                nc.vector.tensor_copy(stg[:, 1024:1028], ksumT)
                ccin = nc.dram_tensor(
                    f"ccin{rep}", [P, 1028], F32
                ).ap()
                ccout = nc.dram_tensor(
                    f"ccout{rep}", [P, 1028], F32
                ).ap()
                nc.sync.dma_start(ccin, stg)
                if not SKIP_CC:
                    nc.gpsimd.collective_compute(
                        "AllReduce",
                        mybir.AluOpType.add,
                        replica_groups=[[0, 1], [2, 3], [4, 5], [6, 7]],
                        ins=[ccin],
                        outs=[ccout],
                    )
                stg2 = pp.tile([P, 1028], F32)
                nc.sync.dma_start(stg2, ccout if not SKIP_CC else ccin)
                kv_src = [
                    stg2[:, 0:512].rearrange("p (a v) -> p a v", a=2),
                    stg2[:, 512:1024].rearrange("p (a v) -> p a v", a=2),
                ]
                ksum_src = stg2[:, 1024:1028]
            else:
                kv_src = kv_ps
                ksum_src = ksumT
            # extract per-head KV blocks into blockdiag layout
            for h in range(H):
                hh = h // 2
                par = h % 2
                vcol = (h % 4) * D
                nc.vector.tensor_copy(
                    kvbd[par * D : (par + 1) * D, hh, h * D : (h + 1) * D],
                    kv_src[hh // 2][par * D : (par + 1) * D, hh % 2,
                                    vcol : vcol + D],
                )
            # blockdiag Ksum [cin_part, cin_tile, H]
            for h in range(H):
                par = h % 2
                ct = h // 2
                nc.vector.tensor_copy(
                    ksbd[par * D : (par + 1) * D, ct, h : h + 1],
                    ksum_src[par * D : (par + 1) * D, ct : ct + 1],
                )

        # ---------------- phase 2: guidance -> Q -> out ----------------
        with (
            tc.tile_pool(name=f"p2_{rep}", bufs=2) as p2,
            tc.tile_pool(name=f"gtp_{rep}", bufs=4) as gtp,
            tc.tile_pool(name=f"p2ps_{rep}", bufs=3, space="PSUM") as p2ps,
            tc.tile_pool(name=f"pops_{rep}", bufs=2, space="PSUM") as pops,
            tc.tile_pool(name=f"tps2_{rep}", bufs=1, space="PSUM") as tps2,
            tc.tile_pool(name=f"dps_{rep}", bufs=2, space="PSUM") as dps,
        ):
            def q_tail(qT, lc):
                # per 128-row l-tile: denominators, then output
                for lt in range(4):
                    lsl = slice(lt * P, (lt + 1) * P)
                    pd = dps.tile([P, H], F32, tag="pd")
                    for ct in range(NCT):
                        mm(pd, qT[:, ct, lsl], ksbd[:, ct, :],
                           start=(ct == 0), stop=(ct == NCT - 1))
                    zl = p2.tile([P, H], F32, tag="zl")
                    nc.vector.tensor_scalar_add(zl, pd, EPS)
                    nc.vector.reciprocal(zl, zl)
                    po = pops.tile([P, C], F32, tag="po")
                    for ct in range(NCT):
                        mm(po, qT[:, ct, lsl], kvbd[:, ct, :],
                           start=(ct == 0), stop=(ct == NCT - 1))
                    osb = p2.tile([P, C], F32, tag="osb")
                    nc.vector.tensor_tensor(
                        osb.rearrange("p (h v) -> p h v", h=H),
                        po.rearrange("p (h v) -> p h v", h=H),
                        zl[:, :, None].to_broadcast([P, H, D]),
                        mybir.AluOpType.mult,
                    )
                    nc.sync.dma_start(
                        outb[(lc * 4 + lt) * P : (lc * 4 + lt + 1) * P, :], osb
                    )

            prev_q = None
            for lc in range(LC // C):
                gT = p2.tile([P, NCT, C], F32R, tag="gT")
                for lt in range(4):
                    gt = gtp.tile([P, C], F32, tag="gt")
                    nc.sync.dma_start(
                        gt, gb[(lc * 4 + lt) * P : (lc * 4 + lt + 1) * P, :]
                    )
                    pt = tps2.tile([P, NCT, P], F32, tag="tp2")
                    for ci in range(NCT):
                        mm(pt[:, ci, :], gt[:, ci * P : (ci + 1) * P], ident,
                           is_transpose=True,
                           start=(ci == 0), stop=(ci == NCT - 1))
                    nc.vector.tensor_copy(gT[:, :, lt * P : (lt + 1) * P], pt)
                # q projection, feature-major: psum[cout, l]
                qT = p2.tile([P, NCT, C], F32R, tag="qT")
                pqs = []
                for ct in range(NCT):
                    pq = p2ps.tile([P, C], F32, tag="pq")
                    for ci in range(NCT):
                        mm(pq, wq_sb[:, ci, ct * P : (ct + 1) * P],
                           gT[:, ci, :], start=(ci == 0), stop=(ci == NCT - 1))
                    pqs.append(pq)
                # previous chunk's tail before this chunk's elu, so the tail
                # DVE/PE work isn't queued behind ACT-dependent elu ops
                if prev_q is not None:
                    q_tail(prev_q, lc - 1)
                for ct in range(NCT):
                    pq = pqs[ct]
                    e2 = p2.tile([P, C], F32, tag="e2")
                    nc.scalar.activation(e2, pq, Exp, bias=bqT[:, ct : ct + 1])
                    u2 = p2.tile([P, C], F32, tag="u2")
                    nc.scalar.activation(u2, pq, Relu, bias=bqT1[:, ct : ct + 1])
                    nc.vector.tensor_scalar_min(e2, e2, 1.0)
                    nc.vector.tensor_tensor(
                        qT[:, ct, :], e2, u2, mybir.AluOpType.max
                    )
                prev_q = qT
            q_tail(prev_q, LC // C - 1)


_CACHE = {}


def _get_nc(reps=1, with_bias=False):
    key = ("nc", reps, with_bias, SPLIT_S, SKIP_CC)
    if key not in _CACHE:
        _CACHE[key] = _build_nc(reps, with_bias, SPLIT_S)
    return _CACHE[key]


def _make_runner(nc):
    """Build a reusable jitted SPMD runner for `nc` (mirrors
    bass2jax.run_bass_via_pjrt's multi-core branch, but caches the jit so
    repeated calls don't re-lower/re-compile)."""
    import jax
    from jax.sharding import Mesh, PartitionSpec
    from jax.experimental.shard_map import shard_map

    import concourse.mybir as mb
    from concourse import bass2jax

    bass2jax.install_neuronx_cc_hook()

    n_cores = 8
    partition_name = (
        nc.partition_id_tensor.name if nc.partition_id_tensor else None
    )
    in_names, out_names, out_avals, zero_shapes = [], [], [], []
    for alloc in nc.m.functions[0].allocations:
        if not isinstance(alloc, mb.MemoryLocationSet):
            continue
        name = alloc.memorylocations[0].name
        if alloc.kind == "ExternalInput":
            if name != partition_name:
                in_names.append(name)
        elif alloc.kind == "ExternalOutput":
            shape = tuple(alloc.tensor_shape)
            dtype = mb.dt.np(alloc.dtype)
            out_names.append(name)
            out_avals.append(jax.core.ShapedArray(shape, dtype))
            zero_shapes.append((shape, dtype))
    n_params = len(in_names)
    n_outs = len(out_names)
    all_names = in_names + out_names
    if partition_name is not None:
        all_names.append(partition_name)
    donate = tuple(range(n_params, n_params + n_outs))

    def _body(*args):
        operands = list(args)
        if partition_name is not None:
            operands.append(bass2jax.partition_id_tensor())
        outs = bass2jax._bass_exec_p.bind(
            *operands,
            out_avals=tuple(out_avals),
            in_names=tuple(all_names),
            out_names=tuple(out_names),
            lowering_input_output_aliases=(),
            sim_require_finite=True,
            sim_require_nnan=True,
            nc=nc,
        )
        return tuple(outs)

    devices = jax.devices()[:n_cores]
    mesh = Mesh(np.asarray(devices), ("core",))
    in_specs = (PartitionSpec("core"),) * (n_params + n_outs)
    out_specs = (PartitionSpec("core"),) * n_outs
    sharded = jax.jit(
        shard_map(
            _body, mesh=mesh, in_specs=in_specs, out_specs=out_specs,
            check_rep=False,
        ),
        donate_argnums=donate,
        keep_unused=True,
    )

    def _zeros():
        return [
            np.zeros((n_cores * sh[0], *sh[1:]), dt) for sh, dt in zero_shapes
        ]

    def runner(concat_in):
        out_arrs = sharded(*concat_in, *_zeros())
        return [
            {
                name: np.asarray(out_arrs[i]).reshape(
                    n_cores, *out_avals[i].shape
                )[c]
                for i, name in enumerate(out_names)
            }
            for c in range(n_cores)
        ]

    def concat(maps):
        return [
            np.concatenate([np.asarray(m[name]) for m in maps], axis=0)
            for name in in_names
        ]

    def timed(concat_in, n=10, warmup=2):
        """Time `n` executions with device-resident inputs and on-device
        donated zero outputs, so per-call host traffic is ~zero."""
        import time as _time
        import jax.numpy as jnp
        from jax.sharding import NamedSharding

        sh = NamedSharding(mesh, PartitionSpec("core"))
        dev_in = [jax.device_put(a, sh) for a in concat_in]

        @jax.jit
        def _mkzeros():
            return tuple(
                jnp.zeros((n_cores * s[0], *s[1:]), d) for s, d in zero_shapes
            )

        _mkzeros = jax.jit(_mkzeros, out_shardings=(sh,) * n_outs)
        times = []
        for i in range(warmup + n):
            z = jax.block_until_ready(_mkzeros())
            t0 = _time.perf_counter()
            outs = sharded(*dev_in, *z)
            jax.block_until_ready(outs)
            dt = _time.perf_counter() - t0
            if i >= warmup:
                times.append(dt)
        return times

    return runner, concat, timed


def _in_maps(x, guidance, Wq, bq, Wk, bk, Wv, bv):
    x = np.ascontiguousarray(x, dtype=np.float32)
    guidance = np.ascontiguousarray(guidance, dtype=np.float32)
    wqt = np.ascontiguousarray(np.asarray(Wq, dtype=np.float32).T)
    wkt = np.ascontiguousarray(np.asarray(Wk, dtype=np.float32).T)
    wvt = np.ascontiguousarray(np.asarray(Wv, dtype=np.float32).T)
    bq = np.ascontiguousarray(bq, dtype=np.float32)
    bk = np.ascontiguousarray(bk, dtype=np.float32)
    bv = np.ascontiguousarray(bv, dtype=np.float32)
    maps = []
    for core in range(8):
        b, half = core // 2, core % 2
        xb_c = (
            x[b, half * (S // 2) : (half + 1) * (S // 2)] if SPLIT_S else x[b]
        )
        maps.append(
            {
                "xb": np.ascontiguousarray(xb_c),
                "gb": np.ascontiguousarray(guidance[b, half * LC : (half + 1) * LC]),
                "wqt": wqt,
                "wkt": wkt,
                "wvt": wvt,
                "bq": bq,
                "bk": bk,
                "bv": bv,
            }
        )
    return maps


def _gather(results):
    B = 4
    out = np.empty((B, 2 * LC, C), dtype=np.float32)
    for core in range(8):
        b, half = core // 2, core % 2
        out[b, half * LC : (half + 1) * LC] = results[core]["outb"]
    return out


def run(inputs, reps=1):
    with_bias = bool(
        np.any(inputs["bq"]) or np.any(inputs["bk"]) or np.any(inputs["bv"])
    )
    nc = _get_nc(reps, with_bias)
    key = ("runner", reps, with_bias, SPLIT_S, SKIP_CC)
    if key not in _CACHE:
        _CACHE[key] = _make_runner(nc)
    runner, concat, timed = _CACHE[key]
    maps = _in_maps(**inputs)
    return runner, timed, concat(maps)


def kernel(**inputs):
    runner, _, concat_in = run(inputs)
    return _gather(runner(concat_in))

